# revision 23
# baseline (speedup 1.0000x reference)
"""DCNv4 block (cv1 1x1 -> offset/mask proj -> deformable bilinear sampling
-> cv2 1x1 -> BN -> SiLU) as a Bass/Tile kernel for Trainium2.

Strategy
--------
Data-parallel over batch: each of the 8 NeuronCores processes one image.

The deformable sampling is reformulated gather-free: with |off| < 1 the
bilinear sample of kernel point k at (h+kh+off_h, w+kw+off_w) equals
  sum_{i,j in {-1,0,1}} tent(off_h - i) * tent(off_w - j) * V[h+kh+i, w+kw+j]
with tent(t) = max(0, 1-|t|).  Merging all 9 kernel points over absolute
displacements e=(eh,ew) in [-2,2]^2 gives 25 "taps":
  out[p,g,:] = sum_e A_e[p,g] * Vpad[p+e, g, :]
  A_e[p,g]   = sum_k mask_k * tent(off_h - (eh-kh)) * tent(off_w - (ew-kw))
Out-of-image corners are handled exactly by zero-padding Vpad (the reference
drops those corners).

Engine mapping:
 - PE: cv1 / offset-projection / cv2 matmuls (float32r), A^T transposes,
   and the 25-term tap accumulation as identity-weight matmuls accumulating
   into PSUM (f32 accumulation).
 - DVE: tent products, A scatter-build, per-tap elementwise A*V products.
 - ACT: tent relus, PSUM->SBUF copies, BN+SiLU epilogue.
 - GPSIMD: a slice of the tap products, memsets.
 - DMA: a replicating access pattern broadcasts per-group tap maps A_e[g,:]
   (16 partitions) to all 128 partitions (V channels are laid out g-major,
   partition j -> group j//8, so one broadcast serves both channel tiles).

All biases ride the matmuls via an appended ones-row.  BN is folded into
cv2 on the host; the offset projection is folded through cv1 on the host so
offsets are computed from x at full precision independent of V's bf16
rounding.
"""

import sys
import numpy as np

if "/opt/trn_rl_repo" not in sys.path:
    sys.path.insert(0, "/opt/trn_rl_repo")

import ml_dtypes

B, C1, C2, H, W = 8, 256, 256, 64, 64
C = 256
G = 16
Cg = 16
K = 9
HW = H * W           # 4096
PW = W + 4           # 68
PH = H + 4
BN_EPS = 1e-5
TPAD = 32            # taps padded to 32 so (t, g) blocks are 128-aligned

_cache = {}


def _v_perm():
    # vtile vt, partition j  ->  original channel g*16 + c
    perm = []
    for vt in range(2):
        for j in range(128):
            g = j // 8
            c = vt * 8 + (j % 8)
            perm.append(g * Cg + c)
    return np.array(perm, np.int64)


def _om_perm():
    # om channel r (0..431) -> original w_off row
    rows = np.zeros(432, np.int64)
    for r in range(144):
        k, g = r // 16, r % 16
        rows[r] = g * 27 + 2 * k            # off_h (dh)
        rows[144 + r] = g * 27 + 2 * k + 1  # off_w (dw)
        rows[288 + r] = g * 27 + 18 + k     # mask
    return rows


def _split_multiwait(nc, mybir, max_waits=1):
    """walrus in this container rejects >1 sem wait on one instruction;
    split extras onto preceding same-engine NoOps (equivalent ordering)."""
    for f in nc.m.functions:
        for bb in f.blocks:
            out = []
            for inst in bb.instructions:
                si = inst.sync_info
                if si is not None and len(si.on_wait) > max_waits:
                    waits = list(si.on_wait)
                    for w in waits[:-max_waits]:
                        nop = mybir.InstNoOp(
                            name=f"I-nopw{nc.next_id()}", ins=[], outs=[])
                        nop.engine = inst.engine
                        nop.sync_info = mybir.SyncInfo(on_wait=[w], on_update=[])
                        nc.register_instruction(nop)
                        out.append(nop)
                    si.on_wait = waits[-max_waits:]
                out.append(inst)
            bb.instructions = out


def _build_nc():
    import concourse.bass as bass
    import concourse.mybir as mybir
    import concourse.tile as tile

    f32 = mybir.dt.float32
    f32r = mybir.dt.float32r
    bf16 = mybir.dt.bfloat16
    ALU = mybir.AluOpType
    ACTF = mybir.ActivationFunctionType

    nc = bass.Bass()

    x_d = nc.dram_tensor("x", [C1, HW], f32r, kind="ExternalInput")
    wt1_d = nc.dram_tensor("wt1", [C1, 256], f32r, kind="ExternalInput")
    wtom_d = nc.dram_tensor("wtom", [C1, 512], f32r, kind="ExternalInput")
    wt2_d = nc.dram_tensor("wt2", [C, C2], bf16, kind="ExternalInput")
    b1_d = nc.dram_tensor("b1", [1, 256], f32r, kind="ExternalInput")
    b2_d = nc.dram_tensor("b2", [C2, 1], f32, kind="ExternalInput")
    bom_d = nc.dram_tensor("bom", [1, 512], f32r, kind="ExternalInput")
    idn_d = nc.dram_tensor("idn", [128, 128], bf16, kind="ExternalInput")
    ones_d = nc.dram_tensor("onesrow", [1, 512], f32r, kind="ExternalInput")
    y_d = nc.dram_tensor("y", [C2, HW], f32, kind="ExternalOutput")

    with tile.TileContext(nc) as tc:
        with tc.tile_pool(name="persist", bufs=1) as persist:

            # ---- persistent tiles ----
            wt1s = [persist.tile([128, 256], f32r, name=f"wt1_{i}") for i in range(2)]
            wtoms = [persist.tile([128, 512], f32r, name=f"wtom_{i}") for i in range(2)]
            wt2s = [persist.tile([128, 256], bf16, name=f"wt2_{i}") for i in range(2)]
            b1row = persist.tile([1, 256], f32r, name="b1row")
            bom1 = persist.tile([1, 512], f32r, name="bom1")
            b2s = [persist.tile([128, 1], f32, name=f"b2_{i}") for i in range(2)]
            ones = persist.tile([1, 512], f32r, name="ones")
            idn = persist.tile([128, 128], bf16, name="idn")
            vpad = [persist.tile([128, PH, PW], bf16, name=f"vpad_{i}") for i in range(2)]
            vpodd = [persist.tile([128, PH * PW], bf16, name=f"vpodd_{i}") for i in range(2)]
            atile = [persist.tile([128, HW], bf16, name=f"atile_{i}") for i in range(4)]
            usb = [persist.tile([128, HW], bf16, name=f"usb_{v}") for v in range(2)]

            for i in range(2):
                nc.sync.dma_start(out=wtoms[i], in_=wtom_d[i * 128:(i + 1) * 128, :])
            for i in range(2):
                nc.sync.dma_start(out=wt1s[i], in_=wt1_d[i * 128:(i + 1) * 128, :])
                nc.sync.dma_start(out=wt2s[i], in_=wt2_d[i * 128:(i + 1) * 128, :])
                nc.sync.dma_start(out=b2s[i], in_=b2_d[i * 128:(i + 1) * 128, :])
            nc.sync.dma_start(out=b1row, in_=b1_d[:, :])
            nc.sync.dma_start(out=bom1, in_=bom_d[:, :])
            nc.sync.dma_start(out=idn, in_=idn_d[:, :])
            nc.sync.dma_start(out=ones, in_=ones_d[:, :])

            # zero the pad ring of Vpad (interior written by cv1)
            for vt in range(2):
                vp = vpad[vt]
                nc.vector.memset(vp[:, 0:2, :], 0.0)
                nc.vector.memset(vp[:, PH - 2:PH, :], 0.0)
                nc.vector.memset(vp[:, 2:PH - 2, 0:2], 0.0)
                nc.vector.memset(vp[:, 2:PH - 2, PW - 4:PW], 0.0)

            with tc.tile_pool(name="build", bufs=1) as bpool, \
                 tc.tile_pool(name="tbuf", bufs=2) as tbuf, \
                 tc.tile_pool(name="ombuf", bufs=2) as ombuf, \
                 tc.tile_pool(name="atbuf", bufs=2) as atbuf, \
                 tc.tile_pool(name="psB", bufs=2, space="PSUM") as psB, \
                 tc.tile_pool(name="trps", bufs=2, space="PSUM") as trps:

                xs = [bpool.tile([128, HW], f32r, name=f"xs_{i}") for i in range(2)]
                for q4 in range(4):
                    for i in range(2):
                        nc.sync.dma_start(
                            out=xs[i][:, q4 * 1024:(q4 + 1) * 1024],
                            in_=x_d[i * 128:(i + 1) * 128, q4 * 1024:(q4 + 1) * 1024])

                # ---- om^T + tents + A-build (chunks of 4 pixel-tiles),
                # interleaved with tap-apply quarters: quarter q consumes the
                # A columns chunks 2q,2q+1 produced, so sampling overlaps the
                # A-map construction ----
                taps = [(eh, ew) for eh in range(-2, 3) for ew in range(-2, 3)]
                if phase < 3:
                    taps = taps[:1]
                QPIX = 1024          # pixels per tap quarter (16 image rows)
                POOL_TAPS = (2, 5, 8, 11, 14, 17, 20, 23)
                cpt = 4
                n_chunk = 8 if phase >= 2 else 0

                def emit_vblock(nt):
                    # cv1 rows nt*8..nt*8+8 for both channel tiles (+bias)
                    for mt in range(2):
                        ps = psB.tile([128, 512], f32, name="omm")
                        for kt in range(2):
                            nc.tensor.matmul(
                                ps, lhsT=wt1s[kt][:, mt * 128:(mt + 1) * 128],
                                rhs=xs[kt][:, nt * 512:(nt + 1) * 512],
                                start=(kt == 0), stop=False)
                        nc.tensor.matmul(
                            ps, lhsT=b1row[0:1, mt * 128:(mt + 1) * 128],
                            rhs=ones[0:1, :], start=False, stop=True)
                        r0v = nt * 8
                        nc.scalar.activation(
                            out=vpad[mt][:, 2 + r0v:2 + r0v + 8, 2:2 + W],
                            in_=ps[:].rearrange("p (r c) -> p r c", c=W),
                            func=ACTF.Copy)

                def emit_vpodd_seg(qq):
                    # vpodd[p, i] = vpad[p, i+1] over padded rows
                    # [qq*16, qq*16+20): covers quarter qq's windows with
                    # 4B-aligned reads for odd-ew taps
                    lo = qq * 16 * PW
                    hi = min((qq * 16 + 20) * PW, PH * PW) - 1
                    for vt in range(2):
                        vflat = vpad[vt][:].rearrange("p a b -> p (a b)")
                        nc.scalar.activation(out=vpodd[vt][:, lo:hi],
                                             in_=vflat[:, lo + 1:hi + 1],
                                             func=ACTF.Copy)

                vblock_sched = {0: [0, 1, 2], 1: [3, 4], 2: [5, 6], 3: [7]}

                with tc.tile_pool(name="abcp", bufs=6) as abcp, \
                     tc.tile_pool(name="prodp", bufs=8) as prodp:

                    for chk in range(n_chunk):
                        om_t = ombuf.tile([128, cpt, 512], f32, name="om_t")
                        for pi in range(cpt):
                            pt = chk * cpt + pi
                            ps = psB.tile([128, 512], f32, name="omm")
                            for kt in range(2):
                                nc.tensor.matmul(
                                    ps, lhsT=xs[kt][:, pt * 128:(pt + 1) * 128],
                                    rhs=wtoms[kt][:, :],
                                    start=(kt == 0), stop=False)
                            nc.tensor.matmul(
                                ps, lhsT=ones[0:1, 0:128],
                                rhs=bom1[0:1, :], start=False, stop=True)
                            nc.scalar.activation(out=om_t[:, pi, :], in_=ps,
                                                 func=ACTF.Copy)

                        oh = om_t[:, :, 0:144]
                        ow = om_t[:, :, 144:288]
                        msk = om_t[:, :, 288:432]

                        th = [tbuf.tile([128, cpt, 144], bf16, name=f"th_{i}") for i in range(3)]
                        tw = [tbuf.tile([128, cpt, 144], bf16, name=f"tw_{i}") for i in range(3)]
                        mbf = tbuf.tile([128, cpt, 144], bf16, name="mbf")

                        # tents (bf16): index 0,1,2 <-> i=-1,0,+1
                        # t(-1)=relu(-o); t(+1)=relu(o); slot1 holds NEGATED
                        # t(0): |o|-1 = relu(o)+relu(-o)-1.  Sign is fixed at
                        # scatter time: terms with exactly one i/j==1 subtract.
                        nc.scalar.activation(out=th[2], in_=oh, func=ACTF.Relu)
                        nc.scalar.activation(out=tw[2], in_=ow, func=ACTF.Relu)
                        nc.scalar.activation(out=th[0], in_=oh, func=ACTF.Relu, scale=-1.0)
                        nc.scalar.activation(out=tw[0], in_=ow, func=ACTF.Relu, scale=-1.0)
                        nc.vector.scalar_tensor_tensor(out=th[1], in0=th[2], scalar=-1.0,
                                                       in1=th[0], op0=ALU.add, op1=ALU.add)
                        nc.vector.scalar_tensor_tensor(out=tw[1], in0=tw[2], scalar=-1.0,
                                                       in1=tw[0], op0=ALU.add, op1=ALU.add)
                        nc.scalar.activation(out=mbf, in_=msk, func=ACTF.Copy)
                        for i in range(3):
                            nc.vector.tensor_tensor(out=th[i], in0=th[i], in1=mbf,
                                                    op=ALU.mult)

                        # A^T chunk [128, cpt, (TPAD t, 16 g)]
                        at = atbuf.tile([128, cpt, TPAD * 16], bf16, name="at")
                        nc.gpsimd.memset(at, 0.0)
                        prod = tbuf.tile([128, cpt, 144], bf16, name="prodb")
                        for i in range(3):
                            for j in range(3):
                                peng = nc.gpsimd if (i, j) in ((0, 0), (2, 2), (0, 2), (2, 0)) else nc.vector
                                peng.tensor_tensor(out=prod, in0=th[i], in1=tw[j],
                                                   op=ALU.mult)
                                a_ap = at[:, :, :]
                                o_ap = bass.AP(
                                    a_ap.tensor,
                                    a_ap.offset + (i * 5 + j) * 16,
                                    [[cpt * TPAD * 16, 128], [TPAD * 16, cpt],
                                     [5 * 16, 3], [1, 48]])
                                p_ap = prod[:, :, :]
                                i_ap = bass.AP(
                                    p_ap.tensor, p_ap.offset,
                                    [[cpt * 144, 128], [144, cpt], [48, 3], [1, 48]])
                                sop = ALU.subtract if (i == 1) != (j == 1) else ALU.add
                                nc.vector.tensor_tensor(out=o_ap, in0=o_ap, in1=i_ap,
                                                        op=sop)

                        # transpose A^T -> A tiles [(t8, g16), pix]
                        for tb in range(4):
                            tps = trps.tile([128, 512], bf16, name="tr")
                            for s in range(4):
                                nc.tensor.transpose(
                                    tps[:, s * 128:(s + 1) * 128],
                                    at[:, s, tb * 128:(tb + 1) * 128],
                                    idn[:, :])
                            col = chk * cpt * 128
                            nc.scalar.activation(
                                out=atile[tb][:, col:col + 512], in_=tps,
                                func=ACTF.Copy)

                        # ---- tap-apply quarter after every odd chunk ----
                        if chk % 2 == 1:
                            for ntv in vblock_sched[chk // 2]:
                                emit_vblock(ntv)
                            emit_vpodd_seg(chk // 2)
                        if phase >= 3 and chk % 2 == 1:
                            qq = chk // 2
                            r0 = qq * 16
                            with tc.tile_pool(name=f"ups{qq}", bufs=1,
                                              space="PSUM") as upsp:
                                ups = [upsp.tile([128, QPIX], f32,
                                                 name=f"ups_{qq}_{v}")
                                       for v in range(2)]
                                for t, (eh, ew) in enumerate(taps):
                                    tb, ts = t // 8, t % 8
                                    abc = abcp.tile([128, QPIX], bf16, name="abc")
                                    a_ap = atile[tb][:, :]
                                    sap = bass.AP(
                                        a_ap.tensor,
                                        a_ap.offset + ts * 16 * HW + qq * QPIX,
                                        [[HW, 16], [0, 8], [1, QPIX]])
                                    nc.sync.dma_start(out=abc, in_=sap)
                                    abc3 = abc[:].rearrange("p (h w) -> p h w", w=W)
                                    for vt in range(2):
                                        if ew % 2 == 0:
                                            win = vpad[vt][:,
                                                           2 + r0 + eh:2 + r0 + eh + 16,
                                                           2 + ew:2 + ew + W]
                                        else:
                                            vp3 = vpodd[vt][:].rearrange(
                                                "p (a b) -> p a b", b=PW)
                                            win = vp3[:,
                                                      2 + r0 + eh:2 + r0 + eh + 16,
                                                      1 + ew:1 + ew + W]
                                        eng = (nc.gpsimd
                                               if (t in POOL_TAPS and vt == 1)
                                               or (t in (6, 12, 18) and vt == 0)
                                               else nc.vector)
                                        pr = prodp.tile([128, QPIX], bf16, name="tp")
                                        eng.tensor_tensor(
                                            out=pr[:].rearrange("p (h w) -> p h w", w=W),
                                            in0=abc3, in1=win, op=ALU.mult)
                                        for nb in range(2):
                                            nc.tensor.matmul(
                                                ups[vt][:, nb * 512:(nb + 1) * 512],
                                                lhsT=idn[:, :],
                                                rhs=pr[:, nb * 512:(nb + 1) * 512],
                                                start=(t == 0),
                                                stop=(t == len(taps) - 1))
                                for vt in range(2):
                                    nc.scalar.activation(
                                        out=usb[vt][:, qq * QPIX:(qq + 1) * QPIX],
                                        in_=ups[vt], func=ACTF.Copy)

                            # cv2 + BN + SiLU for this quarter's pixel columns
                            if phase >= 4:
                                with tc.tile_pool(name=f"cvps{qq}", bufs=2,
                                                  space="PSUM") as cvps, \
                                     tc.tile_pool(name=f"ysb{qq}", bufs=2) as ysbp:
                                    for nt in (2 * qq, 2 * qq + 1):
                                        for mt in range(2):
                                            ps2 = cvps.tile([128, 512], f32,
                                                            name="cv2ps")
                                            for kt in range(2):
                                                nc.tensor.matmul(
                                                    ps2,
                                                    lhsT=wt2s[kt][:, mt * 128:(mt + 1) * 128],
                                                    rhs=usb[kt][:, nt * 512:(nt + 1) * 512],
                                                    start=(kt == 0), stop=(kt == 1))
                                            ysb = ysbp.tile([128, 512], f32,
                                                            name="ysb")
                                            nc.scalar.activation(
                                                out=ysb, in_=ps2, func=ACTF.Silu,
                                                bias=b2s[mt][:, 0:1], scale=1.0)
                                            nc.sync.dma_start(
                                                out=y_d[mt * 128:(mt + 1) * 128,
                                                        nt * 512:(nt + 1) * 512],
                                                in_=ysb)

    _split_multiwait(nc, mybir)
    return nc


def _prepare(inputs):
    x = np.ascontiguousarray(np.asarray(inputs["x"], np.float32))
    w_cv1 = np.asarray(inputs["w_cv1"], np.float32)
    b_cv1 = np.asarray(inputs["b_cv1"], np.float32)
    w_off = np.asarray(inputs["w_off"], np.float32)
    b_off = np.asarray(inputs["b_off"], np.float32)
    w_cv2 = np.asarray(inputs["w_cv2"], np.float32)
    bn_g = np.asarray(inputs["bn_gamma"], np.float32)
    bn_b = np.asarray(inputs["bn_beta"], np.float32)
    bn_m = np.asarray(inputs["bn_mean"], np.float32)
    bn_v = np.asarray(inputs["bn_var"], np.float32)

    perm_v = _v_perm()
    W1p = w_cv1[perm_v, :]
    b1p = b_cv1[perm_v]

    Wom = w_off @ w_cv1
    bom = w_off @ b_cv1 + b_off
    omp = _om_perm()
    Wom_big = np.zeros((512, C1), np.float32)
    Wom_big[:432] = Wom[omp]
    bom_big = np.zeros((512,), np.float32)
    bom_big[:432] = bom[omp]

    s = bn_g / np.sqrt(bn_v + BN_EPS)
    W2s = w_cv2 * s[:, None]
    b2f = bn_b - bn_m * s
    W2p = W2s[:, perm_v]

    shared = dict(
        wt1=np.ascontiguousarray(W1p.T),
        wtom=np.ascontiguousarray(Wom_big.T),
        wt2=np.ascontiguousarray(W2p.T).astype(ml_dtypes.bfloat16),
        b1=np.ascontiguousarray(b1p[None, :]),
        b2=np.ascontiguousarray(b2f[:, None]),
        bom=np.ascontiguousarray(bom_big[None, :]),
        idn=np.eye(128, dtype=ml_dtypes.bfloat16),
        onesrow=np.ones((1, 512), np.float32),
    )
    in_maps = []
    for b in range(B):
        m = dict(shared)
        m["x"] = np.ascontiguousarray(x[b].reshape(C1, HW))
        in_maps.append(m)
    return in_maps


def kernel(**inputs):
    from concourse.bass_utils import run_bass_kernel_spmd

    if "nc" not in _cache:
        _cache["nc"] = _build_nc()
    nc = _cache["nc"]
    in_maps = _prepare(inputs)
    res = run_bass_kernel_spmd(nc, in_maps, core_ids=list(range(B)))
    out = np.stack([r["y"].reshape(C2, H, W) for r in res.results])
    return out.astype(np.float32)


if __name__ == "__main__":
    rng = np.random.default_rng(0)
    demo = dict(
        x=rng.standard_normal((B, C1, H, W)).astype(np.float32),
        w_cv1=rng.standard_normal((C, C1)).astype(np.float32) / 16,
        b_cv1=(rng.standard_normal((C,)) * 0.1).astype(np.float32),
        w_off=(rng.standard_normal((G * 3 * K, C)) * 0.01).astype(np.float32),
        b_off=(rng.standard_normal((G * 3 * K,)) * 0.01).astype(np.float32),
        w_cv2=rng.standard_normal((C2, C)).astype(np.float32) / 16,
        bn_gamma=rng.uniform(0.5, 1.5, (C2,)).astype(np.float32),
        bn_beta=(rng.standard_normal((C2,)) * 0.1).astype(np.float32),
        bn_mean=(rng.standard_normal((C2,)) * 0.1).astype(np.float32),
        bn_var=rng.uniform(0.5, 1.5, (C2,)).astype(np.float32),
    )
    y = kernel(**demo)
    print("kernel ran, output", y.shape, y.dtype)


# revision 29
# speedup vs baseline: 13627.9000x; 13627.9000x over previous
"""DCNv4 block (cv1 1x1 -> offset/mask proj -> deformable bilinear sampling
-> cv2 1x1 -> BN -> SiLU) as a Bass/Tile kernel for Trainium2.

Strategy
--------
Data-parallel over batch: each of the 8 NeuronCores processes one image.

The deformable sampling is reformulated gather-free: with |off| < 1 the
bilinear sample of kernel point k at (h+kh+off_h, w+kw+off_w) equals
  sum_{i,j in {-1,0,1}} tent(off_h - i) * tent(off_w - j) * V[h+kh+i, w+kw+j]
with tent(t) = max(0, 1-|t|).  Merging all 9 kernel points over absolute
displacements e=(eh,ew) in [-2,2]^2 gives 25 "taps":
  out[p,g,:] = sum_e A_e[p,g] * Vpad[p+e, g, :]
  A_e[p,g]   = sum_k mask_k * tent(off_h - (eh-kh)) * tent(off_w - (ew-kw))
Out-of-image corners are handled exactly by zero-padding Vpad (the reference
drops those corners).

Engine mapping:
 - PE: cv1 / offset-projection / cv2 matmuls (float32r), A^T transposes,
   and the 25-term tap accumulation as identity-weight matmuls accumulating
   into PSUM (f32 accumulation).
 - DVE: tent products, A scatter-build, per-tap elementwise A*V products.
 - ACT: tent relus, PSUM->SBUF copies, BN+SiLU epilogue.
 - GPSIMD: a slice of the tap products, memsets.
 - DMA: a replicating access pattern broadcasts per-group tap maps A_e[g,:]
   (16 partitions) to all 128 partitions (V channels are laid out g-major,
   partition j -> group j//8, so one broadcast serves both channel tiles).

All biases ride the matmuls via an appended ones-row.  BN is folded into
cv2 on the host; the offset projection is folded through cv1 on the host so
offsets are computed from x at full precision independent of V's bf16
rounding.
"""

import sys
import numpy as np

if "/opt/trn_rl_repo" not in sys.path:
    sys.path.insert(0, "/opt/trn_rl_repo")

import ml_dtypes

B, C1, C2, H, W = 8, 256, 256, 64, 64
C = 256
G = 16
Cg = 16
K = 9
HW = H * W           # 4096
PW = W + 4           # 68
PH = H + 4
BN_EPS = 1e-5
TPAD = 32            # taps padded to 32 so (t, g) blocks are 128-aligned

_cache = {}


def _v_perm():
    # vtile vt, partition j  ->  original channel g*16 + c
    perm = []
    for vt in range(2):
        for j in range(128):
            g = j // 8
            c = vt * 8 + (j % 8)
            perm.append(g * Cg + c)
    return np.array(perm, np.int64)


def _om_perm():
    # om channel r (0..431) -> original w_off row
    rows = np.zeros(432, np.int64)
    for r in range(144):
        k, g = r // 16, r % 16
        rows[r] = g * 27 + 2 * k            # off_h (dh)
        rows[144 + r] = g * 27 + 2 * k + 1  # off_w (dw)
        rows[288 + r] = g * 27 + 18 + k     # mask
    return rows


def _split_multiwait(nc, mybir, max_waits=1):
    """walrus in this container rejects >1 sem wait on one instruction;
    split extras onto preceding same-engine NoOps (equivalent ordering)."""
    for f in nc.m.functions:
        for bb in f.blocks:
            out = []
            for inst in bb.instructions:
                si = inst.sync_info
                if si is not None and len(si.on_wait) > max_waits:
                    waits = list(si.on_wait)
                    for w in waits[:-max_waits]:
                        nop = mybir.InstNoOp(
                            name=f"I-nopw{nc.next_id()}", ins=[], outs=[])
                        nop.engine = inst.engine
                        nop.sync_info = mybir.SyncInfo(on_wait=[w], on_update=[])
                        nc.register_instruction(nop)
                        out.append(nop)
                    si.on_wait = waits[-max_waits:]
                out.append(inst)
            bb.instructions = out


def _build_nc():
    import concourse.bass as bass
    import concourse.mybir as mybir
    import concourse.tile as tile

    f32 = mybir.dt.float32
    f32r = mybir.dt.float32r
    bf16 = mybir.dt.bfloat16
    ALU = mybir.AluOpType
    ACTF = mybir.ActivationFunctionType

    nc = bass.Bass()

    x_d = nc.dram_tensor("x", [C1, HW], f32r, kind="ExternalInput")
    wt1_d = nc.dram_tensor("wt1", [C1, 256], f32r, kind="ExternalInput")
    wtom_d = nc.dram_tensor("wtom", [C1, 512], f32r, kind="ExternalInput")
    wt2_d = nc.dram_tensor("wt2", [C, C2], bf16, kind="ExternalInput")
    b1_d = nc.dram_tensor("b1", [1, 256], f32r, kind="ExternalInput")
    b2_d = nc.dram_tensor("b2", [C2, 1], f32, kind="ExternalInput")
    bom_d = nc.dram_tensor("bom", [1, 512], f32r, kind="ExternalInput")
    idn_d = nc.dram_tensor("idn", [128, 128], bf16, kind="ExternalInput")
    ones_d = nc.dram_tensor("onesrow", [1, 512], f32r, kind="ExternalInput")
    y_d = nc.dram_tensor("y", [C2, HW], f32, kind="ExternalOutput")

    with tile.TileContext(nc) as tc:
        with tc.tile_pool(name="persist", bufs=1) as persist:

            # ---- persistent tiles ----
            wt1s = [persist.tile([128, 256], f32r, name=f"wt1_{i}") for i in range(2)]
            wtoms = [persist.tile([128, 512], f32r, name=f"wtom_{i}") for i in range(2)]
            wt2s = [persist.tile([128, 256], bf16, name=f"wt2_{i}") for i in range(2)]
            b1row = persist.tile([1, 256], f32r, name="b1row")
            bom1 = persist.tile([1, 512], f32r, name="bom1")
            b2s = [persist.tile([128, 1], f32, name=f"b2_{i}") for i in range(2)]
            ones = persist.tile([1, 512], f32r, name="ones")
            idn = persist.tile([128, 128], bf16, name="idn")
            vpad = [persist.tile([128, PH, PW], bf16, name=f"vpad_{i}") for i in range(2)]
            vpodd = [persist.tile([128, PH * PW], bf16, name=f"vpodd_{i}") for i in range(2)]
            atile = [persist.tile([128, HW], bf16, name=f"atile_{i}") for i in range(4)]
            usb = [persist.tile([128, HW], bf16, name=f"usb_{v}") for v in range(2)]

            for i in range(2):
                nc.sync.dma_start(out=wtoms[i], in_=wtom_d[i * 128:(i + 1) * 128, :])
            for i in range(2):
                nc.sync.dma_start(out=wt1s[i], in_=wt1_d[i * 128:(i + 1) * 128, :])
                nc.sync.dma_start(out=wt2s[i], in_=wt2_d[i * 128:(i + 1) * 128, :])
                nc.sync.dma_start(out=b2s[i], in_=b2_d[i * 128:(i + 1) * 128, :])
            nc.sync.dma_start(out=b1row, in_=b1_d[:, :])
            nc.sync.dma_start(out=bom1, in_=bom_d[:, :])
            nc.sync.dma_start(out=idn, in_=idn_d[:, :])
            nc.sync.dma_start(out=ones, in_=ones_d[:, :])

            # zero the pad ring of Vpad (interior written by cv1)
            for vt in range(2):
                vp = vpad[vt]
                nc.vector.memset(vp[:, 0:2, :], 0.0)
                nc.vector.memset(vp[:, PH - 2:PH, :], 0.0)
                nc.vector.memset(vp[:, 2:PH - 2, 0:2], 0.0)
                nc.vector.memset(vp[:, 2:PH - 2, PW - 4:PW], 0.0)

            with tc.tile_pool(name="build", bufs=1) as bpool, \
                 tc.tile_pool(name="tbuf", bufs=2) as tbuf, \
                 tc.tile_pool(name="ombuf", bufs=2) as ombuf, \
                 tc.tile_pool(name="atbuf", bufs=2) as atbuf, \
                 tc.tile_pool(name="psB", bufs=2, space="PSUM") as psB, \
                 tc.tile_pool(name="trps", bufs=2, space="PSUM") as trps:

                xs = [bpool.tile([128, HW], f32r, name=f"xs_{i}") for i in range(2)]
                for q4 in range(4):
                    for i in range(2):
                        nc.sync.dma_start(
                            out=xs[i][:, q4 * 1024:(q4 + 1) * 1024],
                            in_=x_d[i * 128:(i + 1) * 128, q4 * 1024:(q4 + 1) * 1024])

                # ---- om^T + tents + A-build (chunks of 4 pixel-tiles),
                # interleaved with tap-apply quarters: quarter q consumes the
                # A columns chunks 2q,2q+1 produced, so sampling overlaps the
                # A-map construction ----
                taps = [(eh, ew) for eh in range(-2, 3) for ew in range(-2, 3)]
                if phase < 3:
                    taps = taps[:1]
                QPIX = 1024          # pixels per tap quarter (16 image rows)
                POOL_TAPS = (2, 5, 8, 11, 14, 17, 20, 23)
                cpt = 4
                n_chunk = 8 if phase >= 2 else 0

                def emit_vblock(nt):
                    # cv1 rows nt*8..nt*8+8 for both channel tiles (+bias)
                    for mt in range(2):
                        ps = psB.tile([128, 512], f32, name="omm")
                        for kt in range(2):
                            nc.tensor.matmul(
                                ps, lhsT=wt1s[kt][:, mt * 128:(mt + 1) * 128],
                                rhs=xs[kt][:, nt * 512:(nt + 1) * 512],
                                start=(kt == 0), stop=False)
                        nc.tensor.matmul(
                            ps, lhsT=b1row[0:1, mt * 128:(mt + 1) * 128],
                            rhs=ones[0:1, :], start=False, stop=True)
                        r0v = nt * 8
                        nc.scalar.activation(
                            out=vpad[mt][:, 2 + r0v:2 + r0v + 8, 2:2 + W],
                            in_=ps[:].rearrange("p (r c) -> p r c", c=W),
                            func=ACTF.Copy)

                def emit_vpodd_seg(qq):
                    # vpodd[p, i] = vpad[p, i+1] over padded rows
                    # [qq*16, qq*16+20): covers quarter qq's windows with
                    # 4B-aligned reads for odd-ew taps
                    lo = qq * 16 * PW
                    hi = min((qq * 16 + 20) * PW, PH * PW) - 1
                    for vt in range(2):
                        vflat = vpad[vt][:].rearrange("p a b -> p (a b)")
                        nc.scalar.activation(out=vpodd[vt][:, lo:hi],
                                             in_=vflat[:, lo + 1:hi + 1],
                                             func=ACTF.Copy)

                vblock_sched = {0: [0, 1, 2], 1: [3, 4], 2: [5, 6], 3: [7]}

                with tc.tile_pool(name="abcp", bufs=6) as abcp, \
                     tc.tile_pool(name="prodp", bufs=10) as prodp:

                    for chk in range(n_chunk):
                        om_t = ombuf.tile([128, cpt, 512], f32, name="om_t")
                        for pi in range(cpt):
                            pt = chk * cpt + pi
                            ps = psB.tile([128, 512], f32, name="omm")
                            for kt in range(2):
                                nc.tensor.matmul(
                                    ps, lhsT=xs[kt][:, pt * 128:(pt + 1) * 128],
                                    rhs=wtoms[kt][:, :],
                                    start=(kt == 0), stop=False)
                            nc.tensor.matmul(
                                ps, lhsT=ones[0:1, 0:128],
                                rhs=bom1[0:1, :], start=False, stop=True)
                            nc.scalar.activation(out=om_t[:, pi, :], in_=ps,
                                                 func=ACTF.Copy)

                        oh = om_t[:, :, 0:144]
                        ow = om_t[:, :, 144:288]
                        msk = om_t[:, :, 288:432]

                        th = [tbuf.tile([128, cpt, 144], bf16, name=f"th_{i}") for i in range(3)]
                        tw = [tbuf.tile([128, cpt, 144], bf16, name=f"tw_{i}") for i in range(3)]
                        mbf = tbuf.tile([128, cpt, 144], bf16, name="mbf")

                        # tents (bf16): index 0,1,2 <-> i=-1,0,+1
                        # t(-1)=relu(-o); t(+1)=relu(o); slot1 holds NEGATED
                        # t(0): |o|-1 = relu(o)+relu(-o)-1.  Sign is fixed at
                        # scatter time: terms with exactly one i/j==1 subtract.
                        nc.scalar.activation(out=th[2], in_=oh, func=ACTF.Relu)
                        nc.scalar.activation(out=tw[2], in_=ow, func=ACTF.Relu)
                        nc.scalar.activation(out=th[0], in_=oh, func=ACTF.Relu, scale=-1.0)
                        nc.scalar.activation(out=tw[0], in_=ow, func=ACTF.Relu, scale=-1.0)
                        nc.vector.scalar_tensor_tensor(out=th[1], in0=th[2], scalar=-1.0,
                                                       in1=th[0], op0=ALU.add, op1=ALU.add)
                        nc.vector.scalar_tensor_tensor(out=tw[1], in0=tw[2], scalar=-1.0,
                                                       in1=tw[0], op0=ALU.add, op1=ALU.add)
                        nc.scalar.activation(out=mbf, in_=msk, func=ACTF.Copy)
                        for i in range(3):
                            nc.vector.tensor_tensor(out=th[i], in0=th[i], in1=mbf,
                                                    op=ALU.mult)

                        # A^T chunk [128, cpt, (TPAD t, 16 g)]
                        at = atbuf.tile([128, cpt, TPAD * 16], bf16, name="at")
                        nc.gpsimd.memset(at, 0.0)
                        prod = tbuf.tile([128, cpt, 144], bf16, name="prodb")
                        for i in range(3):
                            for j in range(3):
                                peng = nc.gpsimd if (i, j) in ((0, 0), (2, 2)) else nc.vector
                                peng.tensor_tensor(out=prod, in0=th[i], in1=tw[j],
                                                   op=ALU.mult)
                                a_ap = at[:, :, :]
                                o_ap = bass.AP(
                                    a_ap.tensor,
                                    a_ap.offset + (i * 5 + j) * 16,
                                    [[cpt * TPAD * 16, 128], [TPAD * 16, cpt],
                                     [5 * 16, 3], [1, 48]])
                                p_ap = prod[:, :, :]
                                i_ap = bass.AP(
                                    p_ap.tensor, p_ap.offset,
                                    [[cpt * 144, 128], [144, cpt], [48, 3], [1, 48]])
                                sop = ALU.subtract if (i == 1) != (j == 1) else ALU.add
                                nc.vector.tensor_tensor(out=o_ap, in0=o_ap, in1=i_ap,
                                                        op=sop)

                        # transpose A^T -> A tiles [(t8, g16), pix]
                        for tb in range(4):
                            tps = trps.tile([128, 512], bf16, name="tr")
                            for s in range(4):
                                nc.tensor.transpose(
                                    tps[:, s * 128:(s + 1) * 128],
                                    at[:, s, tb * 128:(tb + 1) * 128],
                                    idn[:, :])
                            col = chk * cpt * 128
                            nc.scalar.activation(
                                out=atile[tb][:, col:col + 512], in_=tps,
                                func=ACTF.Copy)

                        # ---- tap-apply quarter after every odd chunk ----
                        if chk % 2 == 1:
                            for ntv in vblock_sched[chk // 2]:
                                emit_vblock(ntv)
                            emit_vpodd_seg(chk // 2)
                        if phase >= 3 and chk % 2 == 1:
                            qq = chk // 2
                            r0 = qq * 16
                            with tc.tile_pool(name=f"ups{qq}", bufs=1,
                                              space="PSUM") as upsp:
                                ups = [upsp.tile([128, QPIX], f32,
                                                 name=f"ups_{qq}_{v}")
                                       for v in range(2)]
                                for t, (eh, ew) in enumerate(taps):
                                    tb, ts = t // 8, t % 8
                                    abc = abcp.tile([128, QPIX], bf16, name="abc")
                                    a_ap = atile[tb][:, :]
                                    sap = bass.AP(
                                        a_ap.tensor,
                                        a_ap.offset + ts * 16 * HW + qq * QPIX,
                                        [[HW, 16], [0, 8], [1, QPIX]])
                                    nc.sync.dma_start(out=abc, in_=sap)
                                    abc3 = abc[:].rearrange("p (h w) -> p h w", w=W)
                                    for vt in range(2):
                                        if ew % 2 == 0:
                                            win = vpad[vt][:,
                                                           2 + r0 + eh:2 + r0 + eh + 16,
                                                           2 + ew:2 + ew + W]
                                        else:
                                            vp3 = vpodd[vt][:].rearrange(
                                                "p (a b) -> p a b", b=PW)
                                            win = vp3[:,
                                                      2 + r0 + eh:2 + r0 + eh + 16,
                                                      1 + ew:1 + ew + W]
                                        eng = (nc.gpsimd
                                               if (t in POOL_TAPS and vt == 1)
                                               or (t in (6, 18) and vt == 0)
                                               else nc.vector)
                                        pr = prodp.tile([128, QPIX], bf16, name="tp")
                                        eng.tensor_tensor(
                                            out=pr[:].rearrange("p (h w) -> p h w", w=W),
                                            in0=abc3, in1=win, op=ALU.mult)
                                        for nb in range(2):
                                            nc.tensor.matmul(
                                                ups[vt][:, nb * 512:(nb + 1) * 512],
                                                lhsT=idn[:, :],
                                                rhs=pr[:, nb * 512:(nb + 1) * 512],
                                                start=(t == 0),
                                                stop=(t == len(taps) - 1))
                                for vt in range(2):
                                    nc.scalar.activation(
                                        out=usb[vt][:, qq * QPIX:(qq + 1) * QPIX],
                                        in_=ups[vt], func=ACTF.Copy)

                            # cv2 + BN + SiLU for this quarter's pixel columns
                            if phase >= 4:
                                with tc.tile_pool(name=f"cvps{qq}", bufs=2,
                                                  space="PSUM") as cvps, \
                                     tc.tile_pool(name=f"ysb{qq}", bufs=2) as ysbp:
                                    for nt in (2 * qq, 2 * qq + 1):
                                        for mt in range(2):
                                            ps2 = cvps.tile([128, 512], f32,
                                                            name="cv2ps")
                                            for kt in range(2):
                                                nc.tensor.matmul(
                                                    ps2,
                                                    lhsT=wt2s[kt][:, mt * 128:(mt + 1) * 128],
                                                    rhs=usb[kt][:, nt * 512:(nt + 1) * 512],
                                                    start=(kt == 0), stop=(kt == 1))
                                            ysb = ysbp.tile([128, 512], f32,
                                                            name="ysb")
                                            nc.scalar.activation(
                                                out=ysb, in_=ps2, func=ACTF.Silu,
                                                bias=b2s[mt][:, 0:1], scale=1.0)
                                            nc.sync.dma_start(
                                                out=y_d[mt * 128:(mt + 1) * 128,
                                                        nt * 512:(nt + 1) * 512],
                                                in_=ysb)

    _split_multiwait(nc, mybir)
    return nc


def _prepare(inputs):
    x = np.ascontiguousarray(np.asarray(inputs["x"], np.float32))
    w_cv1 = np.asarray(inputs["w_cv1"], np.float32)
    b_cv1 = np.asarray(inputs["b_cv1"], np.float32)
    w_off = np.asarray(inputs["w_off"], np.float32)
    b_off = np.asarray(inputs["b_off"], np.float32)
    w_cv2 = np.asarray(inputs["w_cv2"], np.float32)
    bn_g = np.asarray(inputs["bn_gamma"], np.float32)
    bn_b = np.asarray(inputs["bn_beta"], np.float32)
    bn_m = np.asarray(inputs["bn_mean"], np.float32)
    bn_v = np.asarray(inputs["bn_var"], np.float32)

    perm_v = _v_perm()
    W1p = w_cv1[perm_v, :]
    b1p = b_cv1[perm_v]

    Wom = w_off @ w_cv1
    bom = w_off @ b_cv1 + b_off
    omp = _om_perm()
    Wom_big = np.zeros((512, C1), np.float32)
    Wom_big[:432] = Wom[omp]
    bom_big = np.zeros((512,), np.float32)
    bom_big[:432] = bom[omp]

    s = bn_g / np.sqrt(bn_v + BN_EPS)
    W2s = w_cv2 * s[:, None]
    b2f = bn_b - bn_m * s
    W2p = W2s[:, perm_v]

    shared = dict(
        wt1=np.ascontiguousarray(W1p.T),
        wtom=np.ascontiguousarray(Wom_big.T),
        wt2=np.ascontiguousarray(W2p.T).astype(ml_dtypes.bfloat16),
        b1=np.ascontiguousarray(b1p[None, :]),
        b2=np.ascontiguousarray(b2f[:, None]),
        bom=np.ascontiguousarray(bom_big[None, :]),
        idn=np.eye(128, dtype=ml_dtypes.bfloat16),
        onesrow=np.ones((1, 512), np.float32),
    )
    in_maps = []
    for b in range(B):
        m = dict(shared)
        m["x"] = np.ascontiguousarray(x[b].reshape(C1, HW))
        in_maps.append(m)
    return in_maps


def kernel(**inputs):
    from concourse.bass_utils import run_bass_kernel_spmd

    if "nc" not in _cache:
        _cache["nc"] = _build_nc()
    nc = _cache["nc"]
    in_maps = _prepare(inputs)
    res = run_bass_kernel_spmd(nc, in_maps, core_ids=list(range(B)))
    out = np.stack([r["y"].reshape(C2, H, W) for r in res.results])
    return out.astype(np.float32)


if __name__ == "__main__":
    rng = np.random.default_rng(0)
    demo = dict(
        x=rng.standard_normal((B, C1, H, W)).astype(np.float32),
        w_cv1=rng.standard_normal((C, C1)).astype(np.float32) / 16,
        b_cv1=(rng.standard_normal((C,)) * 0.1).astype(np.float32),
        w_off=(rng.standard_normal((G * 3 * K, C)) * 0.01).astype(np.float32),
        b_off=(rng.standard_normal((G * 3 * K,)) * 0.01).astype(np.float32),
        w_cv2=rng.standard_normal((C2, C)).astype(np.float32) / 16,
        bn_gamma=rng.uniform(0.5, 1.5, (C2,)).astype(np.float32),
        bn_beta=(rng.standard_normal((C2,)) * 0.1).astype(np.float32),
        bn_mean=(rng.standard_normal((C2,)) * 0.1).astype(np.float32),
        bn_var=rng.uniform(0.5, 1.5, (C2,)).astype(np.float32),
    )
    y = kernel(**demo)
    print("kernel ran, output", y.shape, y.dtype)
